# revision 8
# baseline (speedup 1.0000x reference)
"""Trainium2 Bass kernel for nn_CkyLinear: grouped-dequant linear.

reference: W_r = ((W_q - zero) * scale).reshape(4096, 4096); out = x @ W_r.T + bias
  x     [8, 2048, 4096] f32
  W_q   [64, 262144] int32 (u8 codes)
  scale [1, 262144] f32
  zero  [1, 262144] f32
  bias  [4096] f32

Sharding: tensor-parallel over output features, 8 cores x 512 features
(column-parallel linear; x replicated; the op's group layout makes the
scale/zero tables shared by all cores).

Per core: dequantize the W shard on-chip into a resident [4096, 512]
float32r weight, then stream x^T tiles and run float32r matmuls
(lhsT = x^T tile [128i, 128bs] stationary, rhs = W tile [128i, 512o] moving,
psum [128bs, 512o] accumulated over 32 k-tiles). Bias is added by DVE during
PSUM->SBUF eviction. Output shard [16384, 512] f32, host concat over features.

Layout notes:
- x is staged host-side as [t, p, kt, b] (t: 64 bs-tiles of 256, p: 128
  partitions = i%128, kt: 32 k-tiles, b: bs within tile) so each x-tile DMA
  reads one contiguous 32 KiB run per partition (descriptor-cheap, line-rate).
- W_q[g, n] with n = h*4096 + i maps to W_r[o=g*64+h, i]; per-core codes are
  staged partition-major [p, kt, gl*64+h] and fetched in 4 big chunk DMAs;
  scale/zero depend only on (h, i) and are fetched as [p, kt, 64] tables,
  broadcast 8x along the free dim inside the dequant ops.
- Dequant is split 2:1 across DVE and GpSimd so the weight tiles outrun the
  PE's first accumulation group.
- DMA is split across both HWDGE rings (sync + scalar): even x tiles + output
  on sync, odd x tiles + dequant inputs on scalar.
"""
import sys

if "/opt/trn_rl_repo" not in sys.path:
    sys.path.insert(0, "/opt/trn_rl_repo")

import numpy as np

import concourse.bass as bass
import concourse.tile as tile
from concourse import bacc, mybir
from concourse.bass_utils import run_bass_kernel_spmd

B, S, IN_F, OUT_F, GROUP = 8, 2048, 4096, 4096, 64
BS = B * S  # 16384
N_CORES = 8
O_SHARD = OUT_F // N_CORES  # 512
KT = IN_F // 128  # 32 k-tiles
BSB = 256  # bs columns per x tile (2 matmul groups of 128)
N_BST = BS // BSB  # 64
P = 128
KCH = 4  # dequant DMA chunks
KPC = KT // KCH  # 8 k-tiles per chunk

_CACHED_NC = None


def _build():
    nc = bacc.Bacc(trn_type="TRN2", target_bir_lowering=False, debug=False)
    f32 = mybir.dt.float32
    f32r = mybir.dt.float32r

    xt = nc.dram_tensor("xt", [N_BST * P, KT * BSB], f32r, kind="ExternalInput").ap()
    # partition-major weight codes / tables: row p holds [kt, o] / [kt, h]
    wq = nc.dram_tensor("wq", [P, KT * O_SHARD], mybir.dt.uint8, kind="ExternalInput").ap()
    scl = nc.dram_tensor("scl", [P, KT * GROUP], f32, kind="ExternalInput").ap()
    zs = nc.dram_tensor("zs", [P, KT * GROUP], f32, kind="ExternalInput").ap()
    bias_b = nc.dram_tensor("bias_b", [P, O_SHARD], f32, kind="ExternalInput").ap()
    out = nc.dram_tensor("out", [BS, O_SHARD], f32, kind="ExternalOutput").ap()

    xt3 = xt.rearrange("(t p) f -> t p f", p=P)  # [64, 128, 8192]
    wq3 = wq.rearrange("p (c k o) -> p c (k o)", c=KCH, k=KPC)
    scl3 = scl.rearrange("p (c k h) -> p c (k h)", c=KCH, k=KPC)
    zs3 = zs.rearrange("p (c k h) -> p c (k h)", c=KCH, k=KPC)
    out3 = out.rearrange("(t h b) o -> t h b o", h=BSB // P, b=P)

    with tile.TileContext(nc) as tc:
        with (
            tc.tile_pool(name="wres", bufs=1) as wres_pool,
            tc.tile_pool(name="deq", bufs=2) as deq_pool,
            tc.tile_pool(name="bias", bufs=1) as bias_pool,
            tc.tile_pool(name="xin", bufs=3) as x_pool,
            tc.tile_pool(name="psum", bufs=8, space="PSUM") as psum_pool,
            tc.tile_pool(name="oev", bufs=4) as o_pool,
        ):
            bias_sb = bias_pool.tile([P, O_SHARD], f32)
            nc.scalar.dma_start(bias_sb[:], bias_b[:])

            # chunked fetch of dequant inputs (scalar/ACT HWDGE ring)
            wq_ch, sc_ch, zs_ch = [], [], []
            for c in range(KCH):
                wq_t = deq_pool.tile([P, KPC, O_SHARD], mybir.dt.uint8, name="wq_t")
                sc_t = deq_pool.tile([P, KPC, GROUP], f32, name="sc_t")
                zs_t = deq_pool.tile([P, KPC, GROUP], f32, name="zs_t")
                nc.scalar.dma_start(wq_t[:].rearrange("p k o -> p (k o)"), wq3[:, c])
                nc.scalar.dma_start(sc_t[:].rearrange("p k h -> p (k h)"), scl3[:, c])
                nc.scalar.dma_start(zs_t[:].rearrange("p k h -> p (k h)"), zs3[:, c])
                wq_ch.append(wq_t)
                sc_ch.append(sc_t)
                zs_ch.append(zs_t)

            # dequant: w = wq * sc - zs (tables broadcast 8x along free dim)
            w_res = []
            for k in range(KT):
                c, j = divmod(k, KPC)
                w_k = wres_pool.tile([P, O_SHARD], f32r, name=f"w_{k}")
                w_k3 = w_k[:].rearrange("p (g h) -> p g h", h=GROUP)
                wq_k3 = wq_ch[c][:, j, :].rearrange("p (g h) -> p g h", h=GROUP)
                sc_b = sc_ch[c][:, j, None, :].broadcast_to(
                    [P, O_SHARD // GROUP, GROUP]
                )
                zs_b = zs_ch[c][:, j, None, :].broadcast_to(
                    [P, O_SHARD // GROUP, GROUP]
                )
                eng = nc.gpsimd if k % 3 == 2 else nc.vector
                eng.tensor_mul(w_k3, wq_k3, sc_b)
                eng.tensor_sub(w_k3, w_k3, zs_b)
                w_res.append(w_k)

            for t in range(N_BST):
                x_t = x_pool.tile([P, KT, BSB], f32r, name="x_t")
                dma_eng = nc.sync if t % 2 == 0 else nc.scalar
                dma_eng.dma_start(
                    x_t[:], xt3[t].rearrange("p (kt b) -> p kt b", b=BSB)
                )
                for h in range(BSB // P):
                    ps = psum_pool.tile([P, O_SHARD], f32, name="ps")
                    for k in range(KT):
                        nc.tensor.matmul(
                            ps[:],
                            x_t[:, k, bass.ts(h, P)],
                            w_res[k][:],
                            start=(k == 0),
                            stop=(k == KT - 1),
                        )
                    ob = o_pool.tile([P, O_SHARD], f32, name="ob")
                    nc.vector.tensor_add(ob[:], ps[:], bias_sb[:])
                    nc.sync.dma_start(out3[t, h], ob[:])
    nc.compile()
    return nc


def kernel(x, W_q, scale, zero, bias):
    global _CACHED_NC
    if _CACHED_NC is None:
        _CACHED_NC = _build()
    nc = _CACHED_NC

    x = np.asarray(x)
    W_q = np.asarray(W_q)
    scale = np.asarray(scale)
    zero = np.asarray(zero)
    bias = np.asarray(bias)

    # Host-side layout staging (sharding + transposes, no W arithmetic).
    # x[t*256+b, kt*128+p] -> xh[t*128+p, kt*256+b]
    xh = np.ascontiguousarray(
        x.reshape(N_BST, BSB, KT, P).transpose(0, 3, 2, 1).reshape(N_BST * P, KT * BSB)
    ).astype(np.float32, copy=False)
    w3 = W_q.astype(np.uint8).reshape(GROUP, GROUP, IN_F)  # [g, h, i]
    s2 = scale.astype(np.float32).reshape(GROUP, IN_F)  # [h, i]
    zs2 = zero.astype(np.float32).reshape(GROUP, IN_F) * s2  # [h, i]
    # tables partition-major: [i, h] -> [p, kt, h] -> [p, kt*h]
    sclT = np.ascontiguousarray(
        s2.T.reshape(KT, P, GROUP).transpose(1, 0, 2).reshape(P, KT * GROUP)
    )
    zsT = np.ascontiguousarray(
        zs2.T.reshape(KT, P, GROUP).transpose(1, 0, 2).reshape(P, KT * GROUP)
    )

    in_maps = []
    for c in range(N_CORES):
        # codes [i, gl*64+h] -> partition-major [p, kt*(gl*64+h)]
        wq_c = (
            w3[N_CORES * c : N_CORES * (c + 1)]
            .transpose(2, 0, 1)
            .reshape(KT, P, O_SHARD)
            .transpose(1, 0, 2)
            .reshape(P, KT * O_SHARD)
        )
        wq_c = np.ascontiguousarray(wq_c)
        bias_c = bias[O_SHARD * c : O_SHARD * (c + 1)].astype(np.float32)
        bias_bc = np.ascontiguousarray(np.broadcast_to(bias_c, (P, O_SHARD)))
        in_maps.append(
            {"xt": xh, "wq": wq_c, "scl": sclT, "zs": zsT, "bias_b": bias_bc}
        )

    res = run_bass_kernel_spmd(nc, in_maps, core_ids=list(range(N_CORES)))
    out = np.concatenate([res.results[c]["out"] for c in range(N_CORES)], axis=1)
    return out.reshape(B, S, OUT_F)


# revision 9
# speedup vs baseline: 1.0723x; 1.0723x over previous
"""Trainium2 Bass kernel for nn_CkyLinear: grouped-dequant linear.

reference: W_r = ((W_q - zero) * scale).reshape(4096, 4096); out = x @ W_r.T + bias
  x     [8, 2048, 4096] f32
  W_q   [64, 262144] int32 (u8 codes)
  scale [1, 262144] f32
  zero  [1, 262144] f32
  bias  [4096] f32

Sharding: tensor-parallel over output features, 8 cores x 512 features
(column-parallel linear; x replicated; the op's group layout makes the
scale/zero tables shared by all cores).

Per core: dequantize the W shard on-chip into a resident [4096, 512]
float32r weight, then stream x^T tiles and run float32r matmuls
(lhsT = x^T tile [128i, 128bs] stationary, rhs = W tile [128i, 512o] moving,
psum [128bs, 512o] accumulated over 32 k-tiles). Bias is added by DVE during
PSUM->SBUF eviction. Output shard [16384, 512] f32, host concat over features.

Layout notes:
- x is staged host-side as [t, p, kt, b] (t: 64 bs-tiles of 256, p: 128
  partitions = i%128, kt: 32 k-tiles, b: bs within tile) so each x-tile DMA
  reads one contiguous 32 KiB run per partition (descriptor-cheap, line-rate).
- W_q[g, n] with n = h*4096 + i maps to W_r[o=g*64+h, i]; per-core codes are
  staged partition-major [p, kt, gl*64+h] and fetched in 4 big chunk DMAs;
  scale/zero depend only on (h, i) and are fetched as [p, kt, 64] tables,
  broadcast 8x along the free dim inside the dequant ops.
- Dequant is split 2:1 across DVE and GpSimd so the weight tiles outrun the
  PE's first accumulation group.
- DMA is split across both HWDGE rings (sync + scalar): even x tiles + output
  on sync, odd x tiles + dequant inputs on scalar.
"""
import sys

if "/opt/trn_rl_repo" not in sys.path:
    sys.path.insert(0, "/opt/trn_rl_repo")

import numpy as np

import concourse.bass as bass
import concourse.tile as tile
from concourse import bacc, mybir
from concourse.bass_utils import run_bass_kernel_spmd

B, S, IN_F, OUT_F, GROUP = 8, 2048, 4096, 4096, 64
BS = B * S  # 16384
N_CORES = 8
O_SHARD = OUT_F // N_CORES  # 512
KT = IN_F // 128  # 32 k-tiles
BSB = 256  # bs columns per x tile (2 matmul groups of 128)
N_BST = BS // BSB  # 64
P = 128
KCH = 8  # dequant DMA chunks
KPC = KT // KCH  # 8 k-tiles per chunk

_CACHED_NC = None


def _build():
    nc = bacc.Bacc(trn_type="TRN2", target_bir_lowering=False, debug=False)
    f32 = mybir.dt.float32
    f32r = mybir.dt.float32r

    xt = nc.dram_tensor("xt", [N_BST * P, KT * BSB], f32r, kind="ExternalInput").ap()
    # partition-major weight codes / tables: row p holds [kt, o] / [kt, h]
    wq = nc.dram_tensor("wq", [P, KT * O_SHARD], mybir.dt.uint8, kind="ExternalInput").ap()
    scl = nc.dram_tensor("scl", [P, KT * GROUP], f32, kind="ExternalInput").ap()
    zs = nc.dram_tensor("zs", [P, KT * GROUP], f32, kind="ExternalInput").ap()
    bias_b = nc.dram_tensor("bias_b", [P, O_SHARD], f32, kind="ExternalInput").ap()
    out = nc.dram_tensor("out", [BS, O_SHARD], f32, kind="ExternalOutput").ap()

    xt3 = xt.rearrange("(t p) f -> t p f", p=P)  # [64, 128, 8192]
    wq3 = wq.rearrange("p (c k o) -> p c (k o)", c=KCH, k=KPC)
    scl3 = scl.rearrange("p (c k h) -> p c (k h)", c=KCH, k=KPC)
    zs3 = zs.rearrange("p (c k h) -> p c (k h)", c=KCH, k=KPC)
    out3 = out.rearrange("(t h b) o -> t h b o", h=BSB // P, b=P)

    with tile.TileContext(nc) as tc:
        with (
            tc.tile_pool(name="wres", bufs=1) as wres_pool,
            tc.tile_pool(name="deq", bufs=2) as deq_pool,
            tc.tile_pool(name="bias", bufs=1) as bias_pool,
            tc.tile_pool(name="xin", bufs=3) as x_pool,
            tc.tile_pool(name="psum", bufs=8, space="PSUM") as psum_pool,
            tc.tile_pool(name="oev", bufs=4) as o_pool,
        ):
            # chunked fetch of dequant inputs (scalar/ACT HWDGE ring)
            wq_ch, sc_ch, zs_ch = [], [], []
            for c in range(KCH):
                wq_t = deq_pool.tile([P, KPC, O_SHARD], mybir.dt.uint8, name="wq_t")
                sc_t = deq_pool.tile([P, KPC, GROUP], f32, name="sc_t")
                zs_t = deq_pool.tile([P, KPC, GROUP], f32, name="zs_t")
                nc.scalar.dma_start(wq_t[:].rearrange("p k o -> p (k o)"), wq3[:, c])
                nc.scalar.dma_start(sc_t[:].rearrange("p k h -> p (k h)"), scl3[:, c])
                nc.scalar.dma_start(zs_t[:].rearrange("p k h -> p (k h)"), zs3[:, c])
                wq_ch.append(wq_t)
                sc_ch.append(sc_t)
                zs_ch.append(zs_t)

            bias_sb = bias_pool.tile([P, O_SHARD], f32)
            nc.scalar.dma_start(bias_sb[:], bias_b[:])

            # dequant: w = wq * sc - zs (tables broadcast 8x along free dim)
            w_res = []
            for k in range(KT):
                c, j = divmod(k, KPC)
                w_k = wres_pool.tile([P, O_SHARD], f32r, name=f"w_{k}")
                w_k3 = w_k[:].rearrange("p (g h) -> p g h", h=GROUP)
                wq_k3 = wq_ch[c][:, j, :].rearrange("p (g h) -> p g h", h=GROUP)
                sc_b = sc_ch[c][:, j, None, :].broadcast_to(
                    [P, O_SHARD // GROUP, GROUP]
                )
                zs_b = zs_ch[c][:, j, None, :].broadcast_to(
                    [P, O_SHARD // GROUP, GROUP]
                )
                eng = nc.gpsimd if k % 3 == 2 else nc.vector
                eng.tensor_mul(w_k3, wq_k3, sc_b)
                eng.tensor_sub(w_k3, w_k3, zs_b)
                w_res.append(w_k)

            for t in range(N_BST):
                x_t = x_pool.tile([P, KT, BSB], f32r, name="x_t")
                dma_eng = nc.sync if t % 2 == 0 else nc.scalar
                dma_eng.dma_start(
                    x_t[:], xt3[t].rearrange("p (kt b) -> p kt b", b=BSB)
                )
                for h in range(BSB // P):
                    ps = psum_pool.tile([P, O_SHARD], f32, name="ps")
                    for k in range(KT):
                        nc.tensor.matmul(
                            ps[:],
                            x_t[:, k, bass.ts(h, P)],
                            w_res[k][:],
                            start=(k == 0),
                            stop=(k == KT - 1),
                        )
                    ob = o_pool.tile([P, O_SHARD], f32, name="ob")
                    nc.vector.tensor_add(ob[:], ps[:], bias_sb[:])
                    nc.sync.dma_start(out3[t, h], ob[:])
    nc.compile()
    return nc


def kernel(x, W_q, scale, zero, bias):
    global _CACHED_NC
    if _CACHED_NC is None:
        _CACHED_NC = _build()
    nc = _CACHED_NC

    x = np.asarray(x)
    W_q = np.asarray(W_q)
    scale = np.asarray(scale)
    zero = np.asarray(zero)
    bias = np.asarray(bias)

    # Host-side layout staging (sharding + transposes, no W arithmetic).
    # x[t*256+b, kt*128+p] -> xh[t*128+p, kt*256+b]
    xh = np.ascontiguousarray(
        x.reshape(N_BST, BSB, KT, P).transpose(0, 3, 2, 1).reshape(N_BST * P, KT * BSB)
    ).astype(np.float32, copy=False)
    w3 = W_q.astype(np.uint8).reshape(GROUP, GROUP, IN_F)  # [g, h, i]
    s2 = scale.astype(np.float32).reshape(GROUP, IN_F)  # [h, i]
    zs2 = zero.astype(np.float32).reshape(GROUP, IN_F) * s2  # [h, i]
    # tables partition-major: [i, h] -> [p, kt, h] -> [p, kt*h]
    sclT = np.ascontiguousarray(
        s2.T.reshape(KT, P, GROUP).transpose(1, 0, 2).reshape(P, KT * GROUP)
    )
    zsT = np.ascontiguousarray(
        zs2.T.reshape(KT, P, GROUP).transpose(1, 0, 2).reshape(P, KT * GROUP)
    )

    in_maps = []
    for c in range(N_CORES):
        # codes [i, gl*64+h] -> partition-major [p, kt*(gl*64+h)]
        wq_c = (
            w3[N_CORES * c : N_CORES * (c + 1)]
            .transpose(2, 0, 1)
            .reshape(KT, P, O_SHARD)
            .transpose(1, 0, 2)
            .reshape(P, KT * O_SHARD)
        )
        wq_c = np.ascontiguousarray(wq_c)
        bias_c = bias[O_SHARD * c : O_SHARD * (c + 1)].astype(np.float32)
        bias_bc = np.ascontiguousarray(np.broadcast_to(bias_c, (P, O_SHARD)))
        in_maps.append(
            {"xt": xh, "wq": wq_c, "scl": sclT, "zs": zsT, "bias_b": bias_bc}
        )

    res = run_bass_kernel_spmd(nc, in_maps, core_ids=list(range(N_CORES)))
    out = np.concatenate([res.results[c]["out"] for c in range(N_CORES)], axis=1)
    return out.reshape(B, S, OUT_F)
